# revision 24
# baseline (speedup 1.0000x reference)
"""Trainium2 Bass kernel for nn_Event_Critic_Net (dual-branch GAT critic).

Math: the reference only reads the GAT output at the LAST node of each
graph (graphs are 32 contiguous nodes), so only edges whose dst is a
graph's last node contribute.  For those edges the softmax-weighted
aggregation commutes with the linear projection W:

    out_g = sigmoid( (sum_n alpha[n] * x[n,:]) @ W + bias )
    alpha[n] = cnt[n]*exp(e[n]) / (sum_n cnt[n]*exp(e[n]) + 1e-16)
    e[n] = leaky_relu(x[n]. w_src + x[last(g)]. w_dst),  w_* = W @ att_*

cnt[n] = number of edges (n -> last(g(n))).  Per graph only ~7 distinct
source nodes have cnt>0, so the host COMPACTS each graph to K node slots
(zero-padded); GPT graphs share a 128-partition tile -> T tiles per core
instead of 128.  Graph-structure prep (edge counts, gather, tiling,
transposed copy, weight replication) happens on host; all FLOPs on
device.  Sharding: graphs are data-parallel across the 8 cores.

Device pipeline (phases interleaved across branches to keep PE hot):
  logits : xt-chunk [128,128] stationary (FWL), wv2 [128,2] moving
           -> asps psum [128, 2*NCH] (node-layout, 1 bank)
  a_dst  : xl2 mult+reduce -> transpose -> Qm matmul -> adbc [128,T]
  P-chain: z=asps+adbc, e=leakyrelu(DVE), exp(ACT set0), P=e*cnt
  M-build: one tensor_tensor with to_broadcast: M[p,(t,j)]=P[p,t]*Bm[p,j]
  agg    : per tile t: xg-tile [128,66] stationary, M[:,GPT*t..] moving
           -> ynT psum x2; row 64 = denominator (ones column)
  norm   : ACT-Copy evac, recip(DVE) -> rbc = ones64 (x) recip (matmul)
  proj   : ynrm = y*rbc, Wb [64,128] stationary -> h psum [128,512]
  sigmoid: via exp (set0): eu=exp(-h-b); sg_u*sg_d = 1/((1+eu)(1+ed))
  tail   : q=(1+eu)(1+ed), r=recip(q), mlp matmul -> [1,512]+b -> out
"""

import numpy as np
from contextlib import ExitStack

NC = 8            # cores
N = 131072        # nodes total
G = 4096          # graphs
NPG = 32          # nodes per graph
S = 64            # state size
H = 128           # hidden size
GPC = G // NC     # 512 graphs per core
SA = 64           # xg tile cols: 64 feats

_CACHE = {}


def _layout(K):
    GPT = 128 // K               # graphs per tile
    T = -(-GPC // GPT)           # tiles per core
    NT = T * 128                 # slot-rows per core per branch
    XTC = NT // 2                # xt columns
    NCH = XTC // 128             # logit chunks (NT divisible by 256)
    assert NCH * 128 == XTC
    return GPT, T, NT, XTC, NCH


def _build_module(K):
    import concourse.tile as tile
    from concourse import bacc, mybir
    from concourse.alu_op_type import AluOpType as Alu

    GPT, T, NT, XTC, NCH = _layout(K)
    f32 = mybir.dt.float32
    bf16 = mybir.dt.bfloat16
    Act = mybir.ActivationFunctionType
    AxX = mybir.AxisListType.X

    nc = bacc.Bacc("TRN2", target_bir_lowering=False, debug=False,
                   num_devices=NC)

    dram = {}

    def din(name, shape, dt=f32):
        dram[name] = nc.dram_tensor(name, shape, dt, kind="ExternalInput")

    for p in ("u", "d"):
        din(f"{p}_xg", [128, T * SA], bf16)
        din(f"{p}_xt", [128, XTC], bf16)
        din(f"{p}_sm", [128, T + GPT * S], bf16)
    din("cstf", [128, 205])
    din("cstb", [128, 520], bf16)
    out_dram = nc.dram_tensor("out", [1, GPC], f32, kind="ExternalOutput")

    # chunked loads: xt split at 128-col multiples, xg at SA-col multiples
    XT_SPLIT = [0, max(1, NCH // 4), max(2, (5 * NCH) // 8), NCH]
    XG_SPLIT = [0, T // 2, T]

    with tile.TileContext(nc) as tc, ExitStack() as ctx:
        const = ctx.enter_context(tc.tile_pool(name="const", bufs=1))
        xp = ctx.enter_context(tc.tile_pool(name="xp", bufs=2))
        wk = ctx.enter_context(tc.tile_pool(name="wk", bufs=2))
        ps1 = ctx.enter_context(tc.tile_pool(name="ps1", bufs=1, space="PSUM"))
        psA = ctx.enter_context(tc.tile_pool(name="psA", bufs=2, space="PSUM"))
        psY = ctx.enter_context(tc.tile_pool(name="psY", bufs=4, space="PSUM"))

        cstf = const.tile([128, 205], f32, tag="cstf")
        nc.scalar.dma_start(cstf[:], dram["cstf"].ap())
        cstb = const.tile([128, 520], bf16, tag="cstb")
        nc.sync.dma_start(cstb[:], dram["cstb"].ap())
        ident = cstf[:, 0:128]
        eps = cstf[0:1, 128:129]
        mlpb = cstf[0:1, 129:130]
        nbias = {"u": cstf[:, 130:131], "d": cstf[:, 131:132]}  # negated
        Bmf = cstf[:, 132:132 + GPT]
        ones64 = cstf[0:1, 140:204]
        wv2s = {"u": cstb[:, 0:2], "d": cstb[:, 2:4]}
        wdsts = {"u": cstb[:, 4:68], "d": cstb[:, 68:132]}
        Qm = cstb[0:GPT, 132:260]
        Ws = {"u": cstb[0:S, 260:388], "d": cstb[0:S, 388:516]}
        mlpW = cstb[:, 516:517]
        mlpWf = cstf[:, 204:205]

        st = {"u": {}, "d": {}}
        # ---- big loads: u on Sync ring, d on Scalar ring; sm (cnt+xl2)
        # first, then xt chunks, then xg ----
        for p, eng in (("u", nc.sync), ("d", nc.scalar)):
            s = st[p]
            sm = wk.tile([128, T + GPT * S], bf16, tag=f"sm_{p}",
                         name=f"sm_{p}")
            eng.dma_start(sm[:], dram[f"{p}_sm"].ap())
            s["cntb"] = sm[:, 0:T]
            s["xl2"] = sm[:, T:T + GPT * S]
            s["xt"] = []
            for i in range(3):
                w = (XT_SPLIT[i + 1] - XT_SPLIT[i]) * 128
                t_ = xp.tile([128, w], bf16, tag=f"xt{i}", name=f"xt{i}_{p}")
                eng.dma_start(t_[:], dram[f"{p}_xt"].ap()[
                    :, XT_SPLIT[i] * 128:XT_SPLIT[i + 1] * 128])
                s["xt"].append(t_)
            s["xg"] = []
            for i in range(2):
                w = (XG_SPLIT[i + 1] - XG_SPLIT[i]) * SA
                t_ = xp.tile([128, w], bf16, tag=f"xg{i}", name=f"xg{i}_{p}")
                eng.dma_start(t_[:], dram[f"{p}_xg"].ap()[
                    :, XG_SPLIT[i] * SA:XG_SPLIT[i + 1] * SA])
                s["xg"].append(t_)

        for p in ("u", "d"):
            s = st[p]
            cnt = wk.tile([128, T], f32, tag=f"cnt_{p}")
            nc.vector.tensor_copy(cnt[:], s["cntb"])
            s["cnt"] = cnt

        # ---- logits + a_dst chain per branch; P-chain split by xt
        # chunk (host pairs tiles (2c, 2c+1) per xt column chunk) ----
        for p in ("u", "d"):
            s = st[p]
            asps = psA.tile([128, 2 * NCH], f32, tag="asps", name=f"as_{p}")
            s["asps"] = asps
            for c in range(NCH):
                blk = 0
                while XT_SPLIT[blk + 1] <= c:
                    blk += 1
                cc = c - XT_SPLIT[blk]
                nc.tensor.matmul(
                    asps[:, 2 * c:2 * c + 2],
                    s["xt"][blk][:, 128 * cc:128 * cc + 128],
                    wv2s[p],
                    start=True, stop=True)
            tmp6 = wk.tile([128, GPT * S], f32, tag=f"tmp6_{p}")
            nc.vector.tensor_tensor(
                tmp6[:].rearrange("p (j s) -> p j s", s=S),
                s["xl2"].rearrange("p (j s) -> p j s", s=S),
                wdsts[p].unsqueeze(1).to_broadcast((128, GPT, S)),
                op=Alu.mult)
            adst = wk.tile([128, GPT], f32, tag=f"adst_{p}")
            nc.vector.tensor_reduce(
                adst[:], tmp6[:].rearrange("p (j s) -> p j s", s=S),
                axis=AxX, op=Alu.add)
            tp = ps1.tile([GPT, 128], f32, tag="mix", name=f"adT_{p}")
            nc.tensor.transpose(tp[:], adst[:], ident)
            adT = wk.tile([GPT, 128], bf16, tag=f"adTs_{p}")
            nc.vector.tensor_copy(adT[:], tp[:])
            adbc_ps = ps1.tile([128, T], f32, tag="mix", name=f"adbc_{p}")
            nc.tensor.matmul(adbc_ps[:], Qm, adT[:, 0:T],
                             start=True, stop=True)
            adbc = wk.tile([128, T], f32, tag=f"adbcs_{p}")
            nc.vector.tensor_copy(adbc[:], adbc_ps[:])
            s["adbc"] = adbc
            s["z"] = wk.tile([128, T], f32, tag=f"z_{p}", name=f"z_{p}")
            s["e"] = wk.tile([128, T], f32, tag=f"e_{p}", name=f"e_{p}")
            s["ex"] = wk.tile([128, T], f32, tag=f"ex_{p}", name=f"ex_{p}")
            s["P"] = wk.tile([128, T], f32, tag=f"P_{p}", name=f"P_{p}")
            s["M"] = wk.tile([128, T * GPT], bf16, tag=f"M_{p}",
                             name=f"M_{p}")

        def p_chain(p, blk):
            s = st[p]
            t0, t1 = 2 * XT_SPLIT[blk], 2 * XT_SPLIT[blk + 1]
            ts = slice(t0, t1)
            nc.vector.tensor_tensor(
                s["z"][:, ts], s["asps"][:, ts], s["adbc"][:, ts],
                op=Alu.add)
            nc.vector.scalar_tensor_tensor(
                s["e"][:, ts], s["z"][:, ts], 0.2, s["z"][:, ts],
                op0=Alu.mult, op1=Alu.max)
            nc.scalar.activation(s["ex"][:, ts], s["e"][:, ts], Act.Exp)
            nc.vector.tensor_tensor(s["P"][:, ts], s["ex"][:, ts],
                                    s["cnt"][:, ts], op=Alu.mult)
            nc.vector.tensor_tensor(
                s["M"][:].rearrange("p (t j) -> p t j", j=GPT)[:, ts],
                s["P"][:, ts].unsqueeze(2).to_broadcast(
                    (128, t1 - t0, GPT)),
                Bmf.unsqueeze(1).to_broadcast((128, t1 - t0, GPT)),
                op=Alu.mult)

        for p in ("u", "d"):
            p_chain(p, 0)
            p_chain(p, 1)
            p_chain(p, 2)

        # ---- denominators from M (early, off the critical tail):
        # delta[(t,j)] = sum_p M[p,(t,j)]; agg + norm/proj ordered so the
        # post-agg tail has minimal engine crossings ----
        ones128 = cstb[:, 517:518]

        def delta_chain(p):
            s = st[p]
            dps = ps1.tile([1, GPC], f32, tag="mix", name=f"dps_{p}")
            nc.tensor.matmul(dps[:], ones128, s["M"][:, 0:GPC],
                             start=True, stop=True)
            rp = wk.tile([1, GPC], f32, tag=f"rp_{p}", name=f"rp_{p}")
            nc.vector.reciprocal_approx_fast(rp[:], dps[:])
            s["rp"] = rp

        def rbc_mm(p):
            s = st[p]
            rbc = ps1.tile([S, GPC], f32, tag="mix", name=f"rbc_{p}")
            nc.tensor.matmul(rbc[:], ones64, s["rp"][:],
                             start=True, stop=True)
            s["rbc"] = rbc

        def agg_chunk(p, h):
            s = st[p]
            t0, t1 = XG_SPLIT[h], XG_SPLIT[h + 1]
            ynT = psY.tile([S, (t1 - t0) * GPT], f32, tag="ynT",
                           name=f"ynT_{p}{h}")
            for t in range(t0, t1):
                nc.tensor.matmul(
                    ynT[:, GPT * (t - t0):GPT * (t - t0 + 1)],
                    s["xg"][h][:, SA * (t - t0):SA * (t - t0) + SA],
                    s["M"][:, GPT * t:GPT * (t + 1)],
                    start=True, stop=True)
            if p == "d":
                nc.scalar.activation(s["ynsh"][h][:], ynT[:], Act.Copy)
            else:
                nc.scalar.activation(
                    s["yns"][:, t0 * GPT:t1 * GPT], ynT[:], Act.Copy)

        def ynrm_tt(p):
            s = st[p]
            ynrm = wk.tile([S, GPC], bf16, tag=f"ynrm_{p}",
                           name=f"ynrm_{p}")
            nc.vector.tensor_tensor(ynrm[:], s["yns"][:, 0:GPC],
                                    s["rbc"][:], op=Alu.mult)
            s["ynrm"] = ynrm

        def proj_exp(p):
            s = st[p]
            hT = ps1.tile([H, GPC], f32, tag="hT", name=f"hT_{p}")
            nc.tensor.matmul(hT[:], Ws[p], s["ynrm"][:],
                             start=True, stop=True)
            eu = wk.tile([H, GPC], bf16, tag=f"eu_{p}", name=f"eu_{p}")
            nc.scalar.activation(eu[:], hT[:], Act.Exp, bias=nbias[p],
                                 scale=-1.0)
            s["eu"] = eu

        # d-branch tail + combine, pipelined in column halves: half A
        # (cols 0:CA, from agg-d0's yns tile) runs during agg-d1
        CA = (T // 2) * GPT          # 259
        HALVES = [(0, CA), (CA, GPC)]

        def d_half_dve(h):
            lo, hi = HALVES[h]
            w = hi - lo
            s = st["d"]
            ynrm = wk.tile([S, w], bf16, tag=f"ynrmd{h}", name=f"ynrmd{h}")
            nc.vector.tensor_tensor(
                ynrm[:], s["ynsh"][h][:, 0:w], s["rbc"][:, lo:hi],
                op=Alu.mult)
            return ynrm

        def d_half_pre(h, ynrm):
            lo, hi = HALVES[h]
            w = hi - lo
            hT = ps1.tile([H, w], f32, tag="hT", name=f"hTd{h}")
            nc.tensor.matmul(hT[:], Ws["d"], ynrm[:],
                             start=True, stop=True)
            eud = wk.tile([H, w], bf16, tag=f"eud{h}", name=f"eud{h}")
            nc.scalar.activation(eud[:], hT[:], Act.Exp, bias=nbias["d"],
                                 scale=-1.0)
            ed1 = wk.tile([H, w], bf16, tag=f"ed1{h}", name=f"ed1{h}")
            nc.vector.tensor_scalar(
                ed1[:], eud[:], 1.0, None, op0=Alu.add)
            q = wk.tile([H, w], f32, tag=f"q{h}", name=f"q{h}")
            nc.vector.scalar_tensor_tensor(
                q[:], st["u"]["eu"][:, lo:hi], 1.0, ed1[:],
                op0=Alu.add, op1=Alu.mult)
            r32 = wk.tile([H, w], f32, tag=f"r32{h}", name=f"r32{h}")
            nc.vector.reciprocal_approx_fast(r32[:], q[:])
            return r32

        def d_half_post(h, r32):
            lo, hi = HALVES[h]
            w = hi - lo
            o_ps = ps1.tile([1, w], f32, tag="mix", name=f"o_ps{h}")
            nc.tensor.matmul(o_ps[:], mlpWf, r32[:], start=True, stop=True)
            o_sb = wk.tile([1, w], f32, tag=f"o_sb{h}", name=f"o_sb{h}")
            nc.vector.tensor_copy(o_sb[:], o_ps[:])
            nc.sync.dma_start(out_dram.ap()[:, lo:hi], o_sb[:])

        yns_u = wk.tile([S, T * GPT], f32, tag="yns_u", name="yns_u")
        st["u"]["yns"] = yns_u
        ynsh = []
        for h in range(2):
            w = (XG_SPLIT[h + 1] - XG_SPLIT[h]) * GPT
            t_ = wk.tile([S, w], f32, tag=f"yns_d{h}", name=f"yns_d{h}")
            ynsh.append(t_)
        st["d"]["ynsh"] = ynsh
        delta_chain("u")
        agg_chunk("u", 0)
        rbc_mm("u")
        agg_chunk("u", 1)
        delta_chain("d")
        agg_chunk("d", 0)
        ynrm_tt("u")
        rbc_mm("d")
        proj_exp("u")
        ynrm_dA = d_half_dve(0)
        rA = d_half_pre(0, ynrm_dA)
        agg_chunk("d", 1)
        d_half_post(0, rA)
        ynrm_dB = d_half_dve(1)
        rB = d_half_pre(1, ynrm_dB)
        d_half_post(1, rB)

    nc.compile()
    return nc


def _get_module(K):
    key = ("nc", K)
    if key not in _CACHE:
        _CACHE[key] = _build_module(K)
    return _CACHE[key]


def _branch_meta(ei):
    """nodes/counts/slots for one branch (host, structure only)."""
    src = np.asarray(ei[0]).astype(np.int64)
    dst = np.asarray(ei[1]).astype(np.int64)
    valid = (dst % NPG) == (NPG - 1)
    nodes, counts = np.unique(src[valid], return_counts=True)
    gids = nodes // NPG
    order = np.argsort(gids, kind="stable")
    gs = gids[order]
    first = np.r_[True, gs[1:] != gs[:-1]]
    idx_of_first = np.maximum.accumulate(
        np.where(first, np.arange(len(gs)), 0))
    slot_sorted = np.arange(len(gs)) - idx_of_first
    slot = np.empty(len(nodes), np.int64)
    slot[order] = slot_sorted
    maxd = int(slot.max()) + 1 if slot.size else 0
    return nodes, counts, gids, slot, maxd


def _prep_branch(x, W, att_src, att_dst, meta, K):
    import ml_dtypes
    bf = ml_dtypes.bfloat16
    GPT, T, NT, XTC, NCH = _layout(K)
    x = np.asarray(x, np.float32)
    W = np.asarray(W, np.float32)
    w_src = (W @ np.asarray(att_src, np.float32)).astype(np.float32)
    w_dst = (W @ np.asarray(att_dst, np.float32)).astype(np.float32)
    nodes, counts, gids, slot, _ = meta

    per_core = []
    for c in range(NC):
        g_lo, g_hi = c * GPC, (c + 1) * GPC
        m = (gids >= g_lo) & (gids < g_hi)
        nl, cl, gl, sl = nodes[m], counts[m], gids[m] - g_lo, slot[m]
        t = gl // GPT
        part = (gl % GPT) * K + sl
        xg = np.zeros((128, T, SA), np.float32)
        xg[part, t, :S] = x[nl]
        xg2 = np.ascontiguousarray(xg.reshape(128, T * SA)).astype(bf)
        cnt_t = np.zeros((128, T), np.float32)
        cnt_t[part, t] = cl.astype(np.float32)
        # dummy slot-0 weight for graphs with no contributors so the
        # denominator never hits exactly zero (x-row stays 0 -> y = 0)
        has = np.zeros(T * GPT, bool)
        has[gl] = True
        gg_e = np.nonzero(~has[:GPC])[0]
        cnt_t[(gg_e % GPT) * K, gg_e // GPT] = 1.0
        xflat = np.zeros((NT, S), np.float32)
        xflat[t * 128 + part] = x[nl]
        # xt col 128c+m: rows 0:64 = tile 2c row m, rows 64:128 = tile
        # 2c+1 row m  (pairing keeps each xt chunk tile-contiguous)
        xtv = xflat.reshape(T // 2, 2, 128, S).transpose(1, 3, 0, 2)
        xtv = np.ascontiguousarray(xtv.reshape(128, XTC)).astype(bf)
        lg = np.arange(g_lo * NPG + NPG - 1, g_hi * NPG, NPG)
        xl = x[lg].reshape(GPC, S)
        xl2 = np.zeros((128, GPT, S), np.float32)
        gg = np.arange(GPC)
        xl2[gg // GPT, gg % GPT] = xl
        xl2f = np.ascontiguousarray(xl2.reshape(128, GPT * S))
        sm = np.concatenate([cnt_t.astype(np.float32), xl2f], axis=1)
        per_core.append({"xg": xg2, "xt": xtv, "sm": sm.astype(bf)})

    shared = {"w_src": w_src, "w_dst": w_dst, "W": W}
    return per_core, shared


def _build_in_maps(inputs, metas, K):
    import ml_dtypes
    bf = ml_dtypes.bfloat16
    GPT, T, NT, XTC, NCH = _layout(K)
    pcs = {}
    shareds = {}
    pcs["u"], shareds["u"] = _prep_branch(
        inputs["up_x"], inputs["up_W"],
        inputs["up_att_src"], inputs["up_att_dst"], metas["u"], K)
    pcs["d"], shareds["d"] = _prep_branch(
        inputs["down_x"], inputs["down_W"],
        inputs["down_att_src"], inputs["down_att_dst"], metas["d"], K)

    pp = np.arange(128)
    cstf = np.zeros((128, 205), np.float32)
    cstf[:, 0:128] = np.eye(128, dtype=np.float32)
    cstf[0, 128] = 1e-16
    cstf[0, 129] = float(np.asarray(inputs["mlp_b"]).reshape(-1)[0])
    cstf[:, 130] = -np.asarray(inputs["up_bias"], np.float32)
    cstf[:, 131] = -np.asarray(inputs["down_bias"], np.float32)
    # Bmf [128, GPT]: 1 if p//K == j (p < GPT*K)
    cstf[pp[:GPT * K], 132 + pp[:GPT * K] // K] = 1.0
    cstf[0, 140:204] = 1.0
    cstf[:, 204] = np.asarray(inputs["mlp_W"], np.float32).reshape(H)

    cstb = np.zeros((128, 520), np.float32)
    for i, p in enumerate(("u", "d")):
        ws = shareds[p]["w_src"]
        cstb[0:S, 0 + 2 * i] = ws
        cstb[S:128, 1 + 2 * i] = ws
        cstb[:, 4 + S * i:4 + S * (i + 1)] = np.broadcast_to(
            shareds[p]["w_dst"], (128, S))
    # Qm [GPT, 128]: 1 if m//K == j (m < GPT*K)
    for j in range(GPT):
        cstb[j, 132 + j * K:132 + (j + 1) * K] = 1.0
    cstb[0:S, 260:388] = shareds["u"]["W"]
    cstb[0:S, 388:516] = shareds["d"]["W"]
    cstb[:, 516] = np.asarray(inputs["mlp_W"], np.float32).reshape(H)
    cstb[:, 517] = 1.0

    common = {
        "cstf": cstf,
        "cstb": cstb.astype(bf),
    }

    in_maps = []
    for c in range(NC):
        m = dict(common)
        for p in ("u", "d"):
            for k2, v in pcs[p][c].items():
                m[f"{p}_{k2}"] = v
        in_maps.append(m)
    return in_maps


def kernel(**inputs):
    from concourse.bass_utils import run_bass_kernel_spmd

    metas = {"u": _branch_meta(inputs["up_edge_index"]),
             "d": _branch_meta(inputs["down_edge_index"])}
    maxd = max(metas["u"][4], metas["d"][4])
    K = 18 if maxd <= 18 else maxd  # compiled layout adapts to the data
    nc = _get_module(K)
    in_maps = _build_in_maps(inputs, metas, K)
    res = run_bass_kernel_spmd(nc, in_maps, core_ids=list(range(NC)))
    out = np.concatenate(
        [np.asarray(r["out"], np.float32).reshape(GPC) for r in res.results])
    out = out + float(np.asarray(inputs["mlp_b"]).reshape(-1)[0])
    return out.reshape(G, 1)


# revision 25
# speedup vs baseline: 1.1317x; 1.1317x over previous
"""Trainium2 Bass kernel for nn_Event_Critic_Net (dual-branch GAT critic).

Math: the reference only reads the GAT output at the LAST node of each
graph (graphs are 32 contiguous nodes), so only edges whose dst is a
graph's last node contribute.  For those edges the softmax-weighted
aggregation commutes with the linear projection W:

    out_g = sigmoid( (sum_n alpha[n] * x[n,:]) @ W + bias )
    alpha[n] = cnt[n]*exp(e[n]) / (sum_n cnt[n]*exp(e[n]) + 1e-16)
    e[n] = leaky_relu(x[n]. w_src + x[last(g)]. w_dst),  w_* = W @ att_*

cnt[n] = number of edges (n -> last(g(n))).  Per graph only ~7 distinct
source nodes have cnt>0, so the host COMPACTS each graph to K node slots
(zero-padded); GPT graphs share a 128-partition tile -> T tiles per core
instead of 128.  Graph-structure prep (edge counts, gather, tiling,
transposed copy, weight replication) happens on host; all FLOPs on
device.  Sharding: graphs are data-parallel across the 8 cores.

Device pipeline (phases interleaved across branches to keep PE hot):
  logits : xt-chunk [128,128] stationary (FWL), wv2 [128,2] moving
           -> asps psum [128, 2*NCH] (node-layout, 1 bank)
  a_dst  : xl2 mult+reduce -> transpose -> Qm matmul -> adbc [128,T]
  P-chain: z=asps+adbc, e=leakyrelu(DVE), exp(ACT set0), P=e*cnt
  M-build: one tensor_tensor with to_broadcast: M[p,(t,j)]=P[p,t]*Bm[p,j]
  agg    : per tile t: xg-tile [128,66] stationary, M[:,GPT*t..] moving
           -> ynT psum x2; row 64 = denominator (ones column)
  norm   : ACT-Copy evac, recip(DVE) -> rbc = ones64 (x) recip (matmul)
  proj   : ynrm = y*rbc, Wb [64,128] stationary -> h psum [128,512]
  sigmoid: via exp (set0): eu=exp(-h-b); sg_u*sg_d = 1/((1+eu)(1+ed))
  tail   : q=(1+eu)(1+ed), r=recip(q), mlp matmul -> [1,512]+b -> out
"""

import numpy as np
from contextlib import ExitStack

NC = 8            # cores
N = 131072        # nodes total
G = 4096          # graphs
NPG = 32          # nodes per graph
S = 64            # state size
H = 128           # hidden size
GPC = G // NC     # 512 graphs per core
SA = 64           # xg tile cols: 64 feats

_CACHE = {}


def _layout(K):
    GPT = 128 // K               # graphs per tile
    T = -(-GPC // GPT)           # tiles per core
    NT = T * 128                 # slot-rows per core per branch
    XTC = NT // 2                # xt columns
    NCH = XTC // 128             # logit chunks (NT divisible by 256)
    assert NCH * 128 == XTC
    return GPT, T, NT, XTC, NCH


def _build_module(K):
    import concourse.tile as tile
    from concourse import bacc, mybir
    from concourse.alu_op_type import AluOpType as Alu

    GPT, T, NT, XTC, NCH = _layout(K)
    f32 = mybir.dt.float32
    bf16 = mybir.dt.bfloat16
    Act = mybir.ActivationFunctionType
    AxX = mybir.AxisListType.X

    nc = bacc.Bacc("TRN2", target_bir_lowering=False, debug=False,
                   num_devices=NC)

    dram = {}

    def din(name, shape, dt=f32):
        dram[name] = nc.dram_tensor(name, shape, dt, kind="ExternalInput")

    for p in ("u", "d"):
        din(f"{p}_xg", [128, T * SA], bf16)
        din(f"{p}_xt", [128, XTC], bf16)
        din(f"{p}_sm", [128, T + GPT * S], bf16)
    din("cstf", [128, 205])
    din("cstb", [128, 520], bf16)
    out_dram = nc.dram_tensor("out", [1, GPC], f32, kind="ExternalOutput")

    # chunked loads: xt split at 128-col multiples, xg at SA-col multiples
    XT_SPLIT = [0, max(1, NCH // 4), max(2, (5 * NCH) // 8), NCH]
    XG_SPLIT = [0, T // 2, T]

    with tile.TileContext(nc) as tc, ExitStack() as ctx:
        const = ctx.enter_context(tc.tile_pool(name="const", bufs=1))
        xp = ctx.enter_context(tc.tile_pool(name="xp", bufs=2))
        wk = ctx.enter_context(tc.tile_pool(name="wk", bufs=2))
        ps1 = ctx.enter_context(tc.tile_pool(name="ps1", bufs=1, space="PSUM"))
        psA = ctx.enter_context(tc.tile_pool(name="psA", bufs=2, space="PSUM"))
        psY = ctx.enter_context(tc.tile_pool(name="psY", bufs=4, space="PSUM"))

        cstf = const.tile([128, 205], f32, tag="cstf")
        nc.scalar.dma_start(cstf[:], dram["cstf"].ap())
        cstb = const.tile([128, 520], bf16, tag="cstb")
        nc.sync.dma_start(cstb[:], dram["cstb"].ap())
        ident = cstf[:, 0:128]
        eps = cstf[0:1, 128:129]
        mlpb = cstf[0:1, 129:130]
        nbias = {"u": cstf[:, 130:131], "d": cstf[:, 131:132]}  # negated
        Bmf = cstf[:, 132:132 + GPT]
        ones64 = cstf[0:1, 140:204]
        wv2s = {"u": cstb[:, 0:2], "d": cstb[:, 2:4]}
        wdsts = {"u": cstb[:, 4:68], "d": cstb[:, 68:132]}
        Qm = cstb[0:GPT, 132:260]
        Ws = {"u": cstb[0:S, 260:388], "d": cstb[0:S, 388:516]}
        mlpW = cstb[:, 516:517]
        mlpWf = cstf[:, 204:205]

        st = {"u": {}, "d": {}}
        # ---- big loads: u on Sync ring, d on Scalar ring; sm (cnt+xl2)
        # first, then xt chunks, then xg ----
        for p, eng in (("u", nc.sync), ("d", nc.scalar)):
            s = st[p]
            sm = wk.tile([128, T + GPT * S], bf16, tag=f"sm_{p}",
                         name=f"sm_{p}")
            eng.dma_start(sm[:], dram[f"{p}_sm"].ap())
            s["cntb"] = sm[:, 0:T]
            s["xl2"] = sm[:, T:T + GPT * S]
            s["xt"] = []
            for i in range(3):
                w = (XT_SPLIT[i + 1] - XT_SPLIT[i]) * 128
                t_ = xp.tile([128, w], bf16, tag=f"xt{i}", name=f"xt{i}_{p}")
                eng.dma_start(t_[:], dram[f"{p}_xt"].ap()[
                    :, XT_SPLIT[i] * 128:XT_SPLIT[i + 1] * 128])
                s["xt"].append(t_)
            s["xg"] = []
            for i in range(2):
                w = (XG_SPLIT[i + 1] - XG_SPLIT[i]) * SA
                t_ = xp.tile([128, w], bf16, tag=f"xg{i}", name=f"xg{i}_{p}")
                eng.dma_start(t_[:], dram[f"{p}_xg"].ap()[
                    :, XG_SPLIT[i] * SA:XG_SPLIT[i + 1] * SA])
                s["xg"].append(t_)

        for p in ("u", "d"):
            s = st[p]
            cnt = wk.tile([128, T], f32, tag=f"cnt_{p}")
            nc.vector.tensor_copy(cnt[:], s["cntb"])
            s["cnt"] = cnt

        # ---- logits + a_dst chain per branch; P-chain split by xt
        # chunk (host pairs tiles (2c, 2c+1) per xt column chunk) ----
        for p in ("u", "d"):
            s = st[p]
            asps = psA.tile([128, 2 * NCH], f32, tag="asps", name=f"as_{p}")
            s["asps"] = asps
            for c in range(NCH):
                blk = 0
                while XT_SPLIT[blk + 1] <= c:
                    blk += 1
                cc = c - XT_SPLIT[blk]
                nc.tensor.matmul(
                    asps[:, 2 * c:2 * c + 2],
                    s["xt"][blk][:, 128 * cc:128 * cc + 128],
                    wv2s[p],
                    start=True, stop=True)
            tmp6 = wk.tile([128, GPT * S], f32, tag=f"tmp6_{p}")
            nc.vector.tensor_tensor(
                tmp6[:].rearrange("p (j s) -> p j s", s=S),
                s["xl2"].rearrange("p (j s) -> p j s", s=S),
                wdsts[p].unsqueeze(1).to_broadcast((128, GPT, S)),
                op=Alu.mult)
            adst = wk.tile([128, GPT], f32, tag=f"adst_{p}")
            nc.vector.tensor_reduce(
                adst[:], tmp6[:].rearrange("p (j s) -> p j s", s=S),
                axis=AxX, op=Alu.add)
            tp = ps1.tile([GPT, 128], f32, tag="mix", name=f"adT_{p}")
            nc.tensor.transpose(tp[:], adst[:], ident)
            adT = wk.tile([GPT, 128], bf16, tag=f"adTs_{p}")
            nc.vector.tensor_copy(adT[:], tp[:])
            adbc_ps = ps1.tile([128, T], f32, tag="mix", name=f"adbc_{p}")
            nc.tensor.matmul(adbc_ps[:], Qm, adT[:, 0:T],
                             start=True, stop=True)
            adbc = wk.tile([128, T], f32, tag=f"adbcs_{p}")
            nc.vector.tensor_copy(adbc[:], adbc_ps[:])
            s["adbc"] = adbc
            s["z"] = wk.tile([128, T], f32, tag=f"z_{p}", name=f"z_{p}")
            s["e"] = wk.tile([128, T], f32, tag=f"e_{p}", name=f"e_{p}")
            s["ex"] = wk.tile([128, T], f32, tag=f"ex_{p}", name=f"ex_{p}")
            s["P"] = wk.tile([128, T], f32, tag=f"P_{p}", name=f"P_{p}")
            s["M"] = wk.tile([128, T * GPT], bf16, tag=f"M_{p}",
                             name=f"M_{p}")

        def p_chain(p, blk):
            s = st[p]
            t0, t1 = 2 * XT_SPLIT[blk], 2 * XT_SPLIT[blk + 1]
            ts = slice(t0, t1)
            nc.vector.tensor_tensor(
                s["z"][:, ts], s["asps"][:, ts], s["adbc"][:, ts],
                op=Alu.add)
            nc.vector.scalar_tensor_tensor(
                s["e"][:, ts], s["z"][:, ts], 0.2, s["z"][:, ts],
                op0=Alu.mult, op1=Alu.max)
            nc.scalar.activation(s["ex"][:, ts], s["e"][:, ts], Act.Exp)
            nc.vector.tensor_tensor(s["P"][:, ts], s["ex"][:, ts],
                                    s["cnt"][:, ts], op=Alu.mult)
            nc.vector.tensor_tensor(
                s["M"][:].rearrange("p (t j) -> p t j", j=GPT)[:, ts],
                s["P"][:, ts].unsqueeze(2).to_broadcast(
                    (128, t1 - t0, GPT)),
                Bmf.unsqueeze(1).to_broadcast((128, t1 - t0, GPT)),
                op=Alu.mult)

        for p in ("u", "d"):
            p_chain(p, 0)
            p_chain(p, 1)
            p_chain(p, 2)

        # ---- denominators from M (early, off the critical tail):
        # delta[(t,j)] = sum_p M[p,(t,j)]; agg + norm/proj ordered so the
        # post-agg tail has minimal engine crossings ----
        ones128 = cstb[:, 517:518]

        def delta_chain(p):
            s = st[p]
            dps = ps1.tile([1, GPC], f32, tag="mix", name=f"dps_{p}")
            nc.tensor.matmul(dps[:], ones128, s["M"][:, 0:GPC],
                             start=True, stop=True)
            rp = wk.tile([1, GPC], f32, tag=f"rp_{p}", name=f"rp_{p}")
            nc.vector.reciprocal_approx_fast(rp[:], dps[:])
            s["rp"] = rp

        def rbc_mm(p):
            s = st[p]
            rbc = ps1.tile([S, GPC], f32, tag="mix", name=f"rbc_{p}")
            nc.tensor.matmul(rbc[:], ones64, s["rp"][:],
                             start=True, stop=True)
            s["rbc"] = rbc

        def agg_chunk(p, h):
            s = st[p]
            t0, t1 = XG_SPLIT[h], XG_SPLIT[h + 1]
            ynT = psY.tile([S, (t1 - t0) * GPT], f32, tag="ynT",
                           name=f"ynT_{p}{h}")
            for t in range(t0, t1):
                nc.tensor.matmul(
                    ynT[:, GPT * (t - t0):GPT * (t - t0 + 1)],
                    s["xg"][h][:, SA * (t - t0):SA * (t - t0) + SA],
                    s["M"][:, GPT * t:GPT * (t + 1)],
                    start=True, stop=True)
            if p == "d":
                nc.scalar.activation(s["ynsh"][h][:], ynT[:], Act.Copy)
            else:
                nc.scalar.activation(
                    s["yns"][:, t0 * GPT:t1 * GPT], ynT[:], Act.Copy)

        def ynrm_tt(p):
            s = st[p]
            ynrm = wk.tile([S, GPC], bf16, tag=f"ynrm_{p}",
                           name=f"ynrm_{p}")
            nc.vector.tensor_tensor(ynrm[:], s["yns"][:, 0:GPC],
                                    s["rbc"][:], op=Alu.mult)
            s["ynrm"] = ynrm

        def proj_exp(p):
            s = st[p]
            hT = ps1.tile([H, GPC], f32, tag="hT", name=f"hT_{p}")
            nc.tensor.matmul(hT[:], Ws[p], s["ynrm"][:],
                             start=True, stop=True)
            eu = wk.tile([H, GPC], bf16, tag=f"eu_{p}", name=f"eu_{p}")
            nc.scalar.activation(eu[:], hT[:], Act.Exp, bias=nbias[p],
                                 scale=-1.0)
            s["eu"] = eu

        # d-branch tail + combine, pipelined in column halves: half A
        # (cols 0:CA, from agg-d0's yns tile) runs during agg-d1
        CA = (T // 2) * GPT          # 259
        HALVES = [(0, CA), (CA, GPC)]

        def d_half_dve(h):
            lo, hi = HALVES[h]
            w = hi - lo
            s = st["d"]
            ynrm = wk.tile([S, w], bf16, tag=f"ynrmd{h}", name=f"ynrmd{h}")
            nc.vector.tensor_tensor(
                ynrm[:], s["ynsh"][h][:, 0:w], s["rbc"][:, lo:hi],
                op=Alu.mult)
            return ynrm

        def d_half_pre(h, ynrm):
            lo, hi = HALVES[h]
            w = hi - lo
            hT = ps1.tile([H, w], f32, tag="hT", name=f"hTd{h}")
            nc.tensor.matmul(hT[:], Ws["d"], ynrm[:],
                             start=True, stop=True)
            eud = wk.tile([H, w], bf16, tag=f"eud{h}", name=f"eud{h}")
            nc.scalar.activation(eud[:], hT[:], Act.Exp, bias=nbias["d"],
                                 scale=-1.0)
            ed1 = wk.tile([H, w], bf16, tag=f"ed1{h}", name=f"ed1{h}")
            nc.vector.tensor_scalar(
                ed1[:], eud[:], 1.0, None, op0=Alu.add)
            q = wk.tile([H, w], f32, tag=f"q{h}", name=f"q{h}")
            nc.vector.scalar_tensor_tensor(
                q[:], st["u"]["eu"][:, lo:hi], 1.0, ed1[:],
                op0=Alu.add, op1=Alu.mult)
            r32 = wk.tile([H, w], f32, tag=f"r32{h}", name=f"r32{h}")
            nc.vector.reciprocal_approx_fast(r32[:], q[:])
            return r32

        def d_half_post(h, r32):
            lo, hi = HALVES[h]
            w = hi - lo
            o_ps = ps1.tile([1, w], f32, tag="mix", name=f"o_ps{h}")
            nc.tensor.matmul(o_ps[:], mlpWf, r32[:], start=True, stop=True)
            o_sb = wk.tile([1, w], f32, tag=f"o_sb{h}", name=f"o_sb{h}")
            nc.vector.tensor_copy(o_sb[:], o_ps[:])
            nc.sync.dma_start(out_dram.ap()[:, lo:hi], o_sb[:])

        yns_u = wk.tile([S, T * GPT], f32, tag="yns_u", name="yns_u")
        st["u"]["yns"] = yns_u
        ynsh = []
        for h in range(2):
            w = (XG_SPLIT[h + 1] - XG_SPLIT[h]) * GPT
            t_ = wk.tile([S, w], f32, tag=f"yns_d{h}", name=f"yns_d{h}")
            ynsh.append(t_)
        st["d"]["ynsh"] = ynsh
        delta_chain("u")
        agg_chunk("u", 0)
        rbc_mm("u")
        agg_chunk("u", 1)
        delta_chain("d")
        agg_chunk("d", 0)
        ynrm_tt("u")
        rbc_mm("d")
        proj_exp("u")
        ynrm_dA = d_half_dve(0)
        agg_chunk("d", 1)
        rA = d_half_pre(0, ynrm_dA)
        d_half_post(0, rA)
        ynrm_dB = d_half_dve(1)
        rB = d_half_pre(1, ynrm_dB)
        d_half_post(1, rB)

    nc.compile()
    return nc


def _get_module(K):
    key = ("nc", K)
    if key not in _CACHE:
        _CACHE[key] = _build_module(K)
    return _CACHE[key]


def _branch_meta(ei):
    """nodes/counts/slots for one branch (host, structure only)."""
    src = np.asarray(ei[0]).astype(np.int64)
    dst = np.asarray(ei[1]).astype(np.int64)
    valid = (dst % NPG) == (NPG - 1)
    nodes, counts = np.unique(src[valid], return_counts=True)
    gids = nodes // NPG
    order = np.argsort(gids, kind="stable")
    gs = gids[order]
    first = np.r_[True, gs[1:] != gs[:-1]]
    idx_of_first = np.maximum.accumulate(
        np.where(first, np.arange(len(gs)), 0))
    slot_sorted = np.arange(len(gs)) - idx_of_first
    slot = np.empty(len(nodes), np.int64)
    slot[order] = slot_sorted
    maxd = int(slot.max()) + 1 if slot.size else 0
    return nodes, counts, gids, slot, maxd


def _prep_branch(x, W, att_src, att_dst, meta, K):
    import ml_dtypes
    bf = ml_dtypes.bfloat16
    GPT, T, NT, XTC, NCH = _layout(K)
    x = np.asarray(x, np.float32)
    W = np.asarray(W, np.float32)
    w_src = (W @ np.asarray(att_src, np.float32)).astype(np.float32)
    w_dst = (W @ np.asarray(att_dst, np.float32)).astype(np.float32)
    nodes, counts, gids, slot, _ = meta

    per_core = []
    for c in range(NC):
        g_lo, g_hi = c * GPC, (c + 1) * GPC
        m = (gids >= g_lo) & (gids < g_hi)
        nl, cl, gl, sl = nodes[m], counts[m], gids[m] - g_lo, slot[m]
        t = gl // GPT
        part = (gl % GPT) * K + sl
        xg = np.zeros((128, T, SA), np.float32)
        xg[part, t, :S] = x[nl]
        xg2 = np.ascontiguousarray(xg.reshape(128, T * SA)).astype(bf)
        cnt_t = np.zeros((128, T), np.float32)
        cnt_t[part, t] = cl.astype(np.float32)
        # dummy slot-0 weight for graphs with no contributors so the
        # denominator never hits exactly zero (x-row stays 0 -> y = 0)
        has = np.zeros(T * GPT, bool)
        has[gl] = True
        gg_e = np.nonzero(~has[:GPC])[0]
        cnt_t[(gg_e % GPT) * K, gg_e // GPT] = 1.0
        xflat = np.zeros((NT, S), np.float32)
        xflat[t * 128 + part] = x[nl]
        # xt col 128c+m: rows 0:64 = tile 2c row m, rows 64:128 = tile
        # 2c+1 row m  (pairing keeps each xt chunk tile-contiguous)
        xtv = xflat.reshape(T // 2, 2, 128, S).transpose(1, 3, 0, 2)
        xtv = np.ascontiguousarray(xtv.reshape(128, XTC)).astype(bf)
        lg = np.arange(g_lo * NPG + NPG - 1, g_hi * NPG, NPG)
        xl = x[lg].reshape(GPC, S)
        xl2 = np.zeros((128, GPT, S), np.float32)
        gg = np.arange(GPC)
        xl2[gg // GPT, gg % GPT] = xl
        xl2f = np.ascontiguousarray(xl2.reshape(128, GPT * S))
        sm = np.concatenate([cnt_t.astype(np.float32), xl2f], axis=1)
        per_core.append({"xg": xg2, "xt": xtv, "sm": sm.astype(bf)})

    shared = {"w_src": w_src, "w_dst": w_dst, "W": W}
    return per_core, shared


def _build_in_maps(inputs, metas, K):
    import ml_dtypes
    bf = ml_dtypes.bfloat16
    GPT, T, NT, XTC, NCH = _layout(K)
    pcs = {}
    shareds = {}
    pcs["u"], shareds["u"] = _prep_branch(
        inputs["up_x"], inputs["up_W"],
        inputs["up_att_src"], inputs["up_att_dst"], metas["u"], K)
    pcs["d"], shareds["d"] = _prep_branch(
        inputs["down_x"], inputs["down_W"],
        inputs["down_att_src"], inputs["down_att_dst"], metas["d"], K)

    pp = np.arange(128)
    cstf = np.zeros((128, 205), np.float32)
    cstf[:, 0:128] = np.eye(128, dtype=np.float32)
    cstf[0, 128] = 1e-16
    cstf[0, 129] = float(np.asarray(inputs["mlp_b"]).reshape(-1)[0])
    cstf[:, 130] = -np.asarray(inputs["up_bias"], np.float32)
    cstf[:, 131] = -np.asarray(inputs["down_bias"], np.float32)
    # Bmf [128, GPT]: 1 if p//K == j (p < GPT*K)
    cstf[pp[:GPT * K], 132 + pp[:GPT * K] // K] = 1.0
    cstf[0, 140:204] = 1.0
    cstf[:, 204] = np.asarray(inputs["mlp_W"], np.float32).reshape(H)

    cstb = np.zeros((128, 520), np.float32)
    for i, p in enumerate(("u", "d")):
        ws = shareds[p]["w_src"]
        cstb[0:S, 0 + 2 * i] = ws
        cstb[S:128, 1 + 2 * i] = ws
        cstb[:, 4 + S * i:4 + S * (i + 1)] = np.broadcast_to(
            shareds[p]["w_dst"], (128, S))
    # Qm [GPT, 128]: 1 if m//K == j (m < GPT*K)
    for j in range(GPT):
        cstb[j, 132 + j * K:132 + (j + 1) * K] = 1.0
    cstb[0:S, 260:388] = shareds["u"]["W"]
    cstb[0:S, 388:516] = shareds["d"]["W"]
    cstb[:, 516] = np.asarray(inputs["mlp_W"], np.float32).reshape(H)
    cstb[:, 517] = 1.0

    common = {
        "cstf": cstf,
        "cstb": cstb.astype(bf),
    }

    in_maps = []
    for c in range(NC):
        m = dict(common)
        for p in ("u", "d"):
            for k2, v in pcs[p][c].items():
                m[f"{p}_{k2}"] = v
        in_maps.append(m)
    return in_maps


def kernel(**inputs):
    from concourse.bass_utils import run_bass_kernel_spmd

    metas = {"u": _branch_meta(inputs["up_edge_index"]),
             "d": _branch_meta(inputs["down_edge_index"])}
    maxd = max(metas["u"][4], metas["d"][4])
    K = 18 if maxd <= 18 else maxd  # compiled layout adapts to the data
    nc = _get_module(K)
    in_maps = _build_in_maps(inputs, metas, K)
    res = run_bass_kernel_spmd(nc, in_maps, core_ids=list(range(NC)))
    out = np.concatenate(
        [np.asarray(r["out"], np.float32).reshape(GPC) for r in res.results])
    out = out + float(np.asarray(inputs["mlp_b"]).reshape(-1)[0])
    return out.reshape(G, 1)


# revision 26
# speedup vs baseline: 1.1455x; 1.0122x over previous
"""Trainium2 Bass kernel for nn_Event_Critic_Net (dual-branch GAT critic).

Math: the reference only reads the GAT output at the LAST node of each
graph (graphs are 32 contiguous nodes), so only edges whose dst is a
graph's last node contribute.  For those edges the softmax-weighted
aggregation commutes with the linear projection W:

    out_g = sigmoid( (sum_n alpha[n] * x[n,:]) @ W + bias )
    alpha[n] = cnt[n]*exp(e[n]) / (sum_n cnt[n]*exp(e[n]) + 1e-16)
    e[n] = leaky_relu(x[n]. w_src + x[last(g)]. w_dst),  w_* = W @ att_*

cnt[n] = number of edges (n -> last(g(n))).  Per graph only ~7 distinct
source nodes have cnt>0, so the host COMPACTS each graph to K node slots
(zero-padded); GPT graphs share a 128-partition tile -> T tiles per core
instead of 128.  Graph-structure prep (edge counts, gather, tiling,
transposed copy, weight replication) happens on host; all FLOPs on
device.  Sharding: graphs are data-parallel across the 8 cores.

Device pipeline (phases interleaved across branches to keep PE hot):
  logits : xt-chunk [128,128] stationary (FWL), wv2 [128,2] moving
           -> asps psum [128, 2*NCH] (node-layout, 1 bank)
  a_dst  : xl2 mult+reduce -> transpose -> Qm matmul -> adbc [128,T]
  P-chain: z=asps+adbc, e=leakyrelu(DVE), exp(ACT set0), P=e*cnt
  M-build: one tensor_tensor with to_broadcast: M[p,(t,j)]=P[p,t]*Bm[p,j]
  agg    : per tile t: xg-tile [128,66] stationary, M[:,GPT*t..] moving
           -> ynT psum x2; row 64 = denominator (ones column)
  norm   : ACT-Copy evac, recip(DVE) -> rbc = ones64 (x) recip (matmul)
  proj   : ynrm = y*rbc, Wb [64,128] stationary -> h psum [128,512]
  sigmoid: via exp (set0): eu=exp(-h-b); sg_u*sg_d = 1/((1+eu)(1+ed))
  tail   : q=(1+eu)(1+ed), r=recip(q), mlp matmul -> [1,512]+b -> out
"""

import numpy as np
from contextlib import ExitStack

NC = 8            # cores
N = 131072        # nodes total
G = 4096          # graphs
NPG = 32          # nodes per graph
S = 64            # state size
H = 128           # hidden size
GPC = G // NC     # 512 graphs per core
SA = 64           # xg tile cols: 64 feats

_CACHE = {}


def _layout(K):
    GPT = 128 // K               # graphs per tile
    T = -(-GPC // GPT)           # tiles per core
    NT = T * 128                 # slot-rows per core per branch
    XTC = NT // 2                # xt columns
    NCH = XTC // 128             # logit chunks (NT divisible by 256)
    assert NCH * 128 == XTC
    return GPT, T, NT, XTC, NCH


def _build_module(K):
    import concourse.tile as tile
    from concourse import bacc, mybir
    from concourse.alu_op_type import AluOpType as Alu

    GPT, T, NT, XTC, NCH = _layout(K)
    f32 = mybir.dt.float32
    bf16 = mybir.dt.bfloat16
    Act = mybir.ActivationFunctionType
    AxX = mybir.AxisListType.X

    nc = bacc.Bacc("TRN2", target_bir_lowering=False, debug=False,
                   num_devices=NC)

    dram = {}

    def din(name, shape, dt=f32):
        dram[name] = nc.dram_tensor(name, shape, dt, kind="ExternalInput")

    for p in ("u", "d"):
        din(f"{p}_xg", [128, T * SA], bf16)
        din(f"{p}_xt", [128, XTC], bf16)
        din(f"{p}_sm", [128, T + GPT * S], bf16)
    din("cstf", [128, 205])
    din("cstb", [128, 520], bf16)
    out_dram = nc.dram_tensor("out", [1, GPC], f32, kind="ExternalOutput")

    # chunked loads: xt split at 128-col multiples, xg at SA-col multiples
    XT_SPLIT = [0, max(1, NCH // 4), max(2, (5 * NCH) // 8), NCH]
    XG_SPLIT = [0, T // 2, T]

    with tile.TileContext(nc) as tc, ExitStack() as ctx:
        const = ctx.enter_context(tc.tile_pool(name="const", bufs=1))
        xp = ctx.enter_context(tc.tile_pool(name="xp", bufs=2))
        wk = ctx.enter_context(tc.tile_pool(name="wk", bufs=2))
        ps1 = ctx.enter_context(tc.tile_pool(name="ps1", bufs=1, space="PSUM"))
        psA = ctx.enter_context(tc.tile_pool(name="psA", bufs=2, space="PSUM"))
        psY = ctx.enter_context(tc.tile_pool(name="psY", bufs=4, space="PSUM"))

        cstf = const.tile([128, 205], f32, tag="cstf")
        nc.scalar.dma_start(cstf[:], dram["cstf"].ap())
        cstb = const.tile([128, 520], bf16, tag="cstb")
        nc.sync.dma_start(cstb[:], dram["cstb"].ap())
        ident = cstf[:, 0:128]
        eps = cstf[0:1, 128:129]
        mlpb = cstf[0:1, 129:130]
        nbias = {"u": cstf[:, 130:131], "d": cstf[:, 131:132]}  # negated
        Bmf = cstf[:, 132:132 + GPT]
        ones64 = cstf[0:1, 140:204]
        wv2s = {"u": cstb[:, 0:2], "d": cstb[:, 2:4]}
        wdsts = {"u": cstb[:, 4:68], "d": cstb[:, 68:132]}
        Qm = cstb[0:GPT, 132:260]
        Ws = {"u": cstb[0:S, 260:388], "d": cstb[0:S, 388:516]}
        mlpW = cstb[:, 516:517]
        mlpWf = cstf[:, 204:205]

        st = {"u": {}, "d": {}}
        # ---- big loads: u on Sync ring, d on Scalar ring; sm (cnt+xl2)
        # first, then xt chunks, then xg ----
        for p, eng in (("u", nc.sync), ("d", nc.scalar)):
            s = st[p]
            sm = wk.tile([128, T + GPT * S], bf16, tag=f"sm_{p}",
                         name=f"sm_{p}")
            eng.dma_start(sm[:], dram[f"{p}_sm"].ap())
            s["cntb"] = sm[:, 0:T]
            s["xl2"] = sm[:, T:T + GPT * S]
            s["xt"] = []
            for i in range(3):
                w = (XT_SPLIT[i + 1] - XT_SPLIT[i]) * 128
                t_ = xp.tile([128, w], bf16, tag=f"xt{i}", name=f"xt{i}_{p}")
                eng.dma_start(t_[:], dram[f"{p}_xt"].ap()[
                    :, XT_SPLIT[i] * 128:XT_SPLIT[i + 1] * 128])
                s["xt"].append(t_)
            s["xg"] = []
            for i in range(2):
                w = (XG_SPLIT[i + 1] - XG_SPLIT[i]) * SA
                t_ = xp.tile([128, w], bf16, tag=f"xg{i}", name=f"xg{i}_{p}")
                eng.dma_start(t_[:], dram[f"{p}_xg"].ap()[
                    :, XG_SPLIT[i] * SA:XG_SPLIT[i + 1] * SA])
                s["xg"].append(t_)

        for p in ("u", "d"):
            s = st[p]
            cnt = wk.tile([128, T], f32, tag=f"cnt_{p}")
            nc.vector.tensor_copy(cnt[:], s["cntb"])
            s["cnt"] = cnt

        # ---- logits + a_dst chain per branch; P-chain split by xt
        # chunk (host pairs tiles (2c, 2c+1) per xt column chunk) ----
        for p in ("u", "d"):
            s = st[p]
            asps = psA.tile([128, 2 * NCH], f32, tag="asps", name=f"as_{p}")
            s["asps"] = asps
            for c in range(NCH):
                blk = 0
                while XT_SPLIT[blk + 1] <= c:
                    blk += 1
                cc = c - XT_SPLIT[blk]
                nc.tensor.matmul(
                    asps[:, 2 * c:2 * c + 2],
                    s["xt"][blk][:, 128 * cc:128 * cc + 128],
                    wv2s[p],
                    start=True, stop=True)
            tmp6 = wk.tile([128, GPT * S], f32, tag=f"tmp6_{p}")
            nc.vector.tensor_tensor(
                tmp6[:].rearrange("p (j s) -> p j s", s=S),
                s["xl2"].rearrange("p (j s) -> p j s", s=S),
                wdsts[p].unsqueeze(1).to_broadcast((128, GPT, S)),
                op=Alu.mult)
            adst = wk.tile([128, GPT], f32, tag=f"adst_{p}")
            nc.vector.tensor_reduce(
                adst[:], tmp6[:].rearrange("p (j s) -> p j s", s=S),
                axis=AxX, op=Alu.add)
            tp = ps1.tile([GPT, 128], f32, tag="mix", name=f"adT_{p}")
            nc.tensor.transpose(tp[:], adst[:], ident)
            adT = wk.tile([GPT, 128], bf16, tag=f"adTs_{p}")
            nc.vector.tensor_copy(adT[:], tp[:])
            adbc_ps = ps1.tile([128, T], f32, tag="mix", name=f"adbc_{p}")
            nc.tensor.matmul(adbc_ps[:], Qm, adT[:, 0:T],
                             start=True, stop=True)
            adbc = wk.tile([128, T], f32, tag=f"adbcs_{p}")
            nc.vector.tensor_copy(adbc[:], adbc_ps[:])
            s["adbc"] = adbc
            s["z"] = wk.tile([128, T], f32, tag=f"z_{p}", name=f"z_{p}")
            s["e"] = wk.tile([128, T], f32, tag=f"e_{p}", name=f"e_{p}")
            s["ex"] = wk.tile([128, T], f32, tag=f"ex_{p}", name=f"ex_{p}")
            s["P"] = wk.tile([128, T], f32, tag=f"P_{p}", name=f"P_{p}")
            s["M"] = []
            for b in range(3):
                w = (2 * XT_SPLIT[b + 1] - 2 * XT_SPLIT[b]) * GPT
                m_t = wk.tile([128, w], bf16, tag=f"M_{p}{b}",
                              name=f"M_{p}{b}")
                s["M"].append(m_t)

        def p_chain(p, blk):
            s = st[p]
            t0, t1 = 2 * XT_SPLIT[blk], 2 * XT_SPLIT[blk + 1]
            ts = slice(t0, t1)
            nc.vector.tensor_tensor(
                s["z"][:, ts], s["asps"][:, ts], s["adbc"][:, ts],
                op=Alu.add)
            nc.vector.scalar_tensor_tensor(
                s["e"][:, ts], s["z"][:, ts], 0.2, s["z"][:, ts],
                op0=Alu.mult, op1=Alu.max)
            nc.scalar.activation(s["ex"][:, ts], s["e"][:, ts], Act.Exp)
            nc.vector.tensor_tensor(s["P"][:, ts], s["ex"][:, ts],
                                    s["cnt"][:, ts], op=Alu.mult)
            nc.vector.tensor_tensor(
                s["M"][blk][:].rearrange("p (t j) -> p t j", j=GPT),
                s["P"][:, ts].unsqueeze(2).to_broadcast(
                    (128, t1 - t0, GPT)),
                Bmf.unsqueeze(1).to_broadcast((128, t1 - t0, GPT)),
                op=Alu.mult)

        for p in ("u", "d"):
            p_chain(p, 0)
            p_chain(p, 1)
            p_chain(p, 2)

        # ---- denominators from M (early, off the critical tail):
        # delta[(t,j)] = sum_p M[p,(t,j)]; agg + norm/proj ordered so the
        # post-agg tail has minimal engine crossings ----
        ones128 = cstb[:, 517:518]

        def delta_chain(p):
            s = st[p]
            dps = ps1.tile([1, GPC], f32, tag="mix", name=f"dps_{p}")
            for b in range(3):
                lo = 2 * XT_SPLIT[b] * GPT
                hi = min(2 * XT_SPLIT[b + 1] * GPT, GPC)
                nc.tensor.matmul(dps[:, lo:hi], ones128,
                                 s["M"][b][:, 0:hi - lo],
                                 start=True, stop=True)
            rp = wk.tile([1, GPC], f32, tag=f"rp_{p}", name=f"rp_{p}")
            nc.vector.reciprocal_approx_fast(rp[:], dps[:])
            s["rp"] = rp

        def rbc_mm(p):
            s = st[p]
            rbc = ps1.tile([S, GPC], f32, tag="mix", name=f"rbc_{p}")
            nc.tensor.matmul(rbc[:], ones64, s["rp"][:],
                             start=True, stop=True)
            s["rbc"] = rbc

        def agg_chunk(p, h):
            s = st[p]
            t0, t1 = XG_SPLIT[h], XG_SPLIT[h + 1]
            ynT = psY.tile([S, (t1 - t0) * GPT], f32, tag="ynT",
                           name=f"ynT_{p}{h}")
            for t in range(t0, t1):
                mb = 0
                while 2 * XT_SPLIT[mb + 1] <= t:
                    mb += 1
                tloc = t - 2 * XT_SPLIT[mb]
                nc.tensor.matmul(
                    ynT[:, GPT * (t - t0):GPT * (t - t0 + 1)],
                    s["xg"][h][:, SA * (t - t0):SA * (t - t0) + SA],
                    s["M"][mb][:, GPT * tloc:GPT * (tloc + 1)],
                    start=True, stop=True)
            if p == "d":
                nc.scalar.activation(s["ynsh"][h][:], ynT[:], Act.Copy)
            else:
                nc.scalar.activation(
                    s["yns"][:, t0 * GPT:t1 * GPT], ynT[:], Act.Copy)

        def ynrm_tt(p):
            s = st[p]
            ynrm = wk.tile([S, GPC], bf16, tag=f"ynrm_{p}",
                           name=f"ynrm_{p}")
            nc.vector.tensor_tensor(ynrm[:], s["yns"][:, 0:GPC],
                                    s["rbc"][:], op=Alu.mult)
            s["ynrm"] = ynrm

        def proj_exp(p):
            s = st[p]
            hT = ps1.tile([H, GPC], f32, tag="hT", name=f"hT_{p}")
            nc.tensor.matmul(hT[:], Ws[p], s["ynrm"][:],
                             start=True, stop=True)
            eu = wk.tile([H, GPC], bf16, tag=f"eu_{p}", name=f"eu_{p}")
            nc.scalar.activation(eu[:], hT[:], Act.Exp, bias=nbias[p],
                                 scale=-1.0)
            s["eu"] = eu

        # d-branch tail + combine, pipelined in column halves: half A
        # (cols 0:CA, from agg-d0's yns tile) runs during agg-d1
        CA = (T // 2) * GPT          # 259
        HALVES = [(0, CA), (CA, GPC)]

        def d_half_dve(h):
            lo, hi = HALVES[h]
            w = hi - lo
            s = st["d"]
            ynrm = wk.tile([S, w], bf16, tag=f"ynrmd{h}", name=f"ynrmd{h}")
            nc.vector.tensor_tensor(
                ynrm[:], s["ynsh"][h][:, 0:w], s["rbc"][:, lo:hi],
                op=Alu.mult)
            return ynrm

        def d_half_pre(h, ynrm):
            lo, hi = HALVES[h]
            w = hi - lo
            hT = ps1.tile([H, w], f32, tag="hT", name=f"hTd{h}")
            nc.tensor.matmul(hT[:], Ws["d"], ynrm[:],
                             start=True, stop=True)
            eud = wk.tile([H, w], bf16, tag=f"eud{h}", name=f"eud{h}")
            nc.scalar.activation(eud[:], hT[:], Act.Exp, bias=nbias["d"],
                                 scale=-1.0)
            ed1 = wk.tile([H, w], bf16, tag=f"ed1{h}", name=f"ed1{h}")
            nc.vector.tensor_scalar(
                ed1[:], eud[:], 1.0, None, op0=Alu.add)
            q = wk.tile([H, w], f32, tag=f"q{h}", name=f"q{h}")
            nc.vector.scalar_tensor_tensor(
                q[:], st["u"]["eu"][:, lo:hi], 1.0, ed1[:],
                op0=Alu.add, op1=Alu.mult)
            r32 = wk.tile([H, w], f32, tag=f"r32{h}", name=f"r32{h}")
            nc.vector.reciprocal_approx_fast(r32[:], q[:])
            return r32

        def d_half_post(h, r32):
            lo, hi = HALVES[h]
            w = hi - lo
            o_ps = ps1.tile([1, w], f32, tag="mix", name=f"o_ps{h}")
            nc.tensor.matmul(o_ps[:], mlpWf, r32[:], start=True, stop=True)
            o_sb = wk.tile([1, w], f32, tag=f"o_sb{h}", name=f"o_sb{h}")
            nc.vector.tensor_copy(o_sb[:], o_ps[:])
            nc.sync.dma_start(out_dram.ap()[:, lo:hi], o_sb[:])

        yns_u = wk.tile([S, T * GPT], f32, tag="yns_u", name="yns_u")
        st["u"]["yns"] = yns_u
        ynsh = []
        for h in range(2):
            w = (XG_SPLIT[h + 1] - XG_SPLIT[h]) * GPT
            t_ = wk.tile([S, w], f32, tag=f"yns_d{h}", name=f"yns_d{h}")
            ynsh.append(t_)
        st["d"]["ynsh"] = ynsh
        delta_chain("u")
        agg_chunk("u", 0)
        rbc_mm("u")
        agg_chunk("u", 1)
        delta_chain("d")
        agg_chunk("d", 0)
        ynrm_tt("u")
        rbc_mm("d")
        proj_exp("u")
        ynrm_dA = d_half_dve(0)
        agg_chunk("d", 1)
        rA = d_half_pre(0, ynrm_dA)
        d_half_post(0, rA)
        ynrm_dB = d_half_dve(1)
        rB = d_half_pre(1, ynrm_dB)
        d_half_post(1, rB)

    nc.compile()
    return nc


def _get_module(K):
    key = ("nc", K)
    if key not in _CACHE:
        _CACHE[key] = _build_module(K)
    return _CACHE[key]


def _branch_meta(ei):
    """nodes/counts/slots for one branch (host, structure only)."""
    src = np.asarray(ei[0]).astype(np.int64)
    dst = np.asarray(ei[1]).astype(np.int64)
    valid = (dst % NPG) == (NPG - 1)
    nodes, counts = np.unique(src[valid], return_counts=True)
    gids = nodes // NPG
    order = np.argsort(gids, kind="stable")
    gs = gids[order]
    first = np.r_[True, gs[1:] != gs[:-1]]
    idx_of_first = np.maximum.accumulate(
        np.where(first, np.arange(len(gs)), 0))
    slot_sorted = np.arange(len(gs)) - idx_of_first
    slot = np.empty(len(nodes), np.int64)
    slot[order] = slot_sorted
    maxd = int(slot.max()) + 1 if slot.size else 0
    return nodes, counts, gids, slot, maxd


def _prep_branch(x, W, att_src, att_dst, meta, K):
    import ml_dtypes
    bf = ml_dtypes.bfloat16
    GPT, T, NT, XTC, NCH = _layout(K)
    x = np.asarray(x, np.float32)
    W = np.asarray(W, np.float32)
    w_src = (W @ np.asarray(att_src, np.float32)).astype(np.float32)
    w_dst = (W @ np.asarray(att_dst, np.float32)).astype(np.float32)
    nodes, counts, gids, slot, _ = meta

    per_core = []
    for c in range(NC):
        g_lo, g_hi = c * GPC, (c + 1) * GPC
        m = (gids >= g_lo) & (gids < g_hi)
        nl, cl, gl, sl = nodes[m], counts[m], gids[m] - g_lo, slot[m]
        t = gl // GPT
        part = (gl % GPT) * K + sl
        xg = np.zeros((128, T, SA), np.float32)
        xg[part, t, :S] = x[nl]
        xg2 = np.ascontiguousarray(xg.reshape(128, T * SA)).astype(bf)
        cnt_t = np.zeros((128, T), np.float32)
        cnt_t[part, t] = cl.astype(np.float32)
        # dummy slot-0 weight for graphs with no contributors so the
        # denominator never hits exactly zero (x-row stays 0 -> y = 0)
        has = np.zeros(T * GPT, bool)
        has[gl] = True
        gg_e = np.nonzero(~has[:GPC])[0]
        cnt_t[(gg_e % GPT) * K, gg_e // GPT] = 1.0
        xflat = np.zeros((NT, S), np.float32)
        xflat[t * 128 + part] = x[nl]
        # xt col 128c+m: rows 0:64 = tile 2c row m, rows 64:128 = tile
        # 2c+1 row m  (pairing keeps each xt chunk tile-contiguous)
        xtv = xflat.reshape(T // 2, 2, 128, S).transpose(1, 3, 0, 2)
        xtv = np.ascontiguousarray(xtv.reshape(128, XTC)).astype(bf)
        lg = np.arange(g_lo * NPG + NPG - 1, g_hi * NPG, NPG)
        xl = x[lg].reshape(GPC, S)
        xl2 = np.zeros((128, GPT, S), np.float32)
        gg = np.arange(GPC)
        xl2[gg // GPT, gg % GPT] = xl
        xl2f = np.ascontiguousarray(xl2.reshape(128, GPT * S))
        sm = np.concatenate([cnt_t.astype(np.float32), xl2f], axis=1)
        per_core.append({"xg": xg2, "xt": xtv, "sm": sm.astype(bf)})

    shared = {"w_src": w_src, "w_dst": w_dst, "W": W}
    return per_core, shared


def _build_in_maps(inputs, metas, K):
    import ml_dtypes
    bf = ml_dtypes.bfloat16
    GPT, T, NT, XTC, NCH = _layout(K)
    pcs = {}
    shareds = {}
    pcs["u"], shareds["u"] = _prep_branch(
        inputs["up_x"], inputs["up_W"],
        inputs["up_att_src"], inputs["up_att_dst"], metas["u"], K)
    pcs["d"], shareds["d"] = _prep_branch(
        inputs["down_x"], inputs["down_W"],
        inputs["down_att_src"], inputs["down_att_dst"], metas["d"], K)

    pp = np.arange(128)
    cstf = np.zeros((128, 205), np.float32)
    cstf[:, 0:128] = np.eye(128, dtype=np.float32)
    cstf[0, 128] = 1e-16
    cstf[0, 129] = float(np.asarray(inputs["mlp_b"]).reshape(-1)[0])
    cstf[:, 130] = -np.asarray(inputs["up_bias"], np.float32)
    cstf[:, 131] = -np.asarray(inputs["down_bias"], np.float32)
    # Bmf [128, GPT]: 1 if p//K == j (p < GPT*K)
    cstf[pp[:GPT * K], 132 + pp[:GPT * K] // K] = 1.0
    cstf[0, 140:204] = 1.0
    cstf[:, 204] = np.asarray(inputs["mlp_W"], np.float32).reshape(H)

    cstb = np.zeros((128, 520), np.float32)
    for i, p in enumerate(("u", "d")):
        ws = shareds[p]["w_src"]
        cstb[0:S, 0 + 2 * i] = ws
        cstb[S:128, 1 + 2 * i] = ws
        cstb[:, 4 + S * i:4 + S * (i + 1)] = np.broadcast_to(
            shareds[p]["w_dst"], (128, S))
    # Qm [GPT, 128]: 1 if m//K == j (m < GPT*K)
    for j in range(GPT):
        cstb[j, 132 + j * K:132 + (j + 1) * K] = 1.0
    cstb[0:S, 260:388] = shareds["u"]["W"]
    cstb[0:S, 388:516] = shareds["d"]["W"]
    cstb[:, 516] = np.asarray(inputs["mlp_W"], np.float32).reshape(H)
    cstb[:, 517] = 1.0

    common = {
        "cstf": cstf,
        "cstb": cstb.astype(bf),
    }

    in_maps = []
    for c in range(NC):
        m = dict(common)
        for p in ("u", "d"):
            for k2, v in pcs[p][c].items():
                m[f"{p}_{k2}"] = v
        in_maps.append(m)
    return in_maps


def kernel(**inputs):
    from concourse.bass_utils import run_bass_kernel_spmd

    metas = {"u": _branch_meta(inputs["up_edge_index"]),
             "d": _branch_meta(inputs["down_edge_index"])}
    maxd = max(metas["u"][4], metas["d"][4])
    K = 18 if maxd <= 18 else maxd  # compiled layout adapts to the data
    nc = _get_module(K)
    in_maps = _build_in_maps(inputs, metas, K)
    res = run_bass_kernel_spmd(nc, in_maps, core_ids=list(range(NC)))
    out = np.concatenate(
        [np.asarray(r["out"], np.float32).reshape(GPC) for r in res.results])
    out = out + float(np.asarray(inputs["mlp_b"]).reshape(-1)[0])
    return out.reshape(G, 1)
